# revision 6
# baseline (speedup 1.0000x reference)
"""Cost-volume kernel (nn_CostVolume) for Trainium2, 8 NeuronCores — v2.

out[b, i, h, w] = mean_c feat1[b, c, h, w] * feat2[b, c, h, w + i - 4]
(feat2 zero-padded along width), inputs (8, 256, 96, 320) fp32,
output (8, 9, 96, 320) fp32.

Strategy
--------
Data-parallel over B: core b handles batch b (communication-free).

Per core, for each (h, 64-wide w-block) the 9 shifted channel-dot-products
are computed on the TensorEngine as a banded correlation matmul

    band[p, n] = sum_c f1[c, w0+p] * f2[c, w0-4+n],   p in [0,64), n in [0,72)

with the C=256 contraction split into two PSUM-accumulated K=128 matmuls.
The 9 useful diagonals band[p, p+i] are extracted by dumping the bands to
flat-addressed HBM scratch and gathering them back with a (row+1)-strided
DMA, then DVE-stream-transposing [w, (i, h)] -> [h, (i, w)].

v2 changes vs the first working version (356 us):
 * one input DMA per (chunk, tensor) with both 128-channel halves folded
   into the free dim — 24 big contiguous loads instead of 48 (f2's were
   strided); f2 zero-padding is gone: edge blocks run subrange matmuls and
   the out-of-range band columns are memset once per half.
 * band kept in bf16 (tolerance is 2e-2; bf16 quantization of the final
   means is ~4e-3 relative) — halves the scratch round-trip traffic.
 * queue separation: SP issues ONLY the input loads, Pool/SWDGE issues the
   tail DMAs (dump+gather+output, merged to one each per half), ACT does
   only psum->band copies.  DMA instructions that must wait (gather on
   dump, output on transposes) now stall only the idle Pool queue instead
   of head-of-line-blocking the input stream, which was the main source of
   DMA idle time.
 * the [w, (i, h)] -> [h, (i, w)] transpose runs on the DVE as ten 32x32
   stream-transposes per slice whose free-dim APs drop each shift-block
   into its output slot directly — no PSUM tiles, no psum->sbuf copies,
   and the PE stream is pure matmuls (previously the PE-transpose tail
   head-of-line-blocked the next slice's matmuls for ~16 us per slice
   while waiting on the gather).  The staging tile stays bf16 and the
   output DMA casts back to fp32 in flight (SWDGE).

Measured (reps-slope on axon-tunneled trn2): ~0.19-0.21 ms per core vs
0.23-0.26 ms for the previous version in the same process; TimelineSim
puts the steady-state rep at 190 us with the DMA engines 96% busy, i.e.
at the HBM roofline for the ~69 MB/core of traffic (63 MB fp32 features
+ 5 MB bf16 scratch round-trip + 1 MB output).
"""

import numpy as np

import concourse.bacc as bacc
import concourse.bass as bass
import concourse.tile as tile
from concourse import mybir
from concourse.bass_utils import run_bass_kernel_spmd

B, C, H, W = 8, 256, 96, 320
D = 4
NS = 2 * D + 1  # 9 shifts
P = 128  # partitions per c-block
M = 64  # w-block size
NB = W // M  # 5 w-blocks
NBAND = M + 2 * D  # 72 band columns
NH = 8  # h rows per feature chunk
NCHUNK = H // NH  # 12
NHALF = 3  # image slices for tail pipelining
H2 = H // NHALF  # 32 h rows per slice
CPH = NCHUNK // NHALF  # 4 chunks per slice

F32 = mybir.dt.float32
BF16 = mybir.dt.bfloat16

_cache: dict = {}


def _build(reps: int = 1, skip_gather: bool = False, skip_compute: bool = False,
           skip_mm: bool = False, skip_act: bool = False):
    nc = bacc.Bacc("TRN2", target_bir_lowering=False, debug=False, num_devices=B)
    f1 = nc.dram_tensor("f1", (C, H, W), F32, kind="ExternalInput")
    f2 = nc.dram_tensor("f2", (C, H, W), F32, kind="ExternalInput")
    out = nc.dram_tensor("out", (NS, H, W), F32, kind="ExternalOutput")

    with tile.TileContext(nc) as tc:
        with (
            tc.tile_pool(name="feat", bufs=3) as fpool,
            tc.tile_pool(name="band", bufs=1) as bpool,
            tc.tile_pool(name="gat", bufs=2) as gpool,
            tc.tile_pool(name="osb", bufs=2) as opool,
            tc.tile_pool(name="ps", bufs=8, space="PSUM") as pspool,
            tc.tile_pool(name="scratch", bufs=1, space="DRAM") as dpool,
        ):
            pools = (fpool, bpool, gpool, opool, pspool, dpool)
            # state: running slice index (for global band/scratch tag
            # alternation) and the slice whose tail is still pending — tails
            # are deferred past the next slice's first chunk of matmuls so
            # the PE queue never stalls on the dump->gather chain.
            state = {"g": 0, "pending": None}
            for _rep in range(reps):
                _body(
                    nc, tc, pools, f1, f2, out, state,
                    skip_gather=skip_gather, skip_compute=skip_compute,
                    skip_mm=skip_mm, skip_act=skip_act,
                )
            if state["pending"] is not None:
                _transpose_out(nc, pools, out, *state["pending"])

    nc.compile()
    return nc


def _dump_gather(nc, pools, band, g):
    """Dump one slice's bands to HBM scratch and gather the diagonals back.
    Pool/SWDGE only — emitted immediately when the slice's bands are done."""
    fpool, bpool, gpool, opool, pspool, dpool = pools

    scratch = dpool.tile([M, NB, NBAND, H2], BF16, tag=f"scr{g % 2}")
    nc.gpsimd.dma_start(
        out=scratch.rearrange("p a n h -> p (a n h)"),
        in_=band.rearrange("p a n h -> p (a n h)"),
    )

    # The 9 diagonals of row p are columns [p, p+8]: with h innermost these
    # are 9*H2 consecutive scratch elements per partition-row, so a single
    # (row+1)-strided DMA gathers every (block, shift).
    sc_p = NB * NBAND * H2  # scratch partition-row length in elements
    g9 = gpool.tile([M, NB, NS, H2], BF16, tag="g9")
    src = bass.AP(
        tensor=scratch.tensor,
        offset=scratch.offset,
        ap=[[sc_p + H2, M], [NBAND * H2, NB], [1, NS * H2]],
    )
    with nc.allow_non_contiguous_dma("banded diagonal gather"):
        nc.gpsimd.dma_start(out=g9.rearrange("p a i h -> p (a i h)"), in_=src)
    return g9


def _transpose_out(nc, pools, out, g9, half):
    """Transpose [w, (i, h)] -> [h, (i, w)] with DVE 32x32 stream transposes
    (the free-dim AP drops each shift-block into its osb slot), then one
    casting output DMA.  No PE or PSUM involvement — the tail never blocks
    the matmul stream."""
    fpool, bpool, gpool, opool, pspool, dpool = pools

    osb = opool.tile([H2, NS, W], BF16, tag="osb")
    for blk in range(NB):
        for i2 in range(M // 32):
            w0 = blk * M + 32 * i2
            nc.vector.transpose(
                osb[:, :, w0 : w0 + 32],
                g9[32 * i2 : 32 * i2 + 32, blk, :, :],
            )
    dst = bass.AP(
        tensor=out.ap().tensor,
        offset=half * H2 * W,
        ap=[[W, H2], [H * W, NS], [1, W]],
    )
    # SWDGE casts the bf16 staging tile back to the fp32 output during DMA.
    nc.gpsimd.dma_start(out=dst, in_=osb.rearrange("h i w -> h (i w)"))


def _body(nc, tc, pools, f1, f2, out, state,
          skip_gather=False, skip_compute=False, skip_mm=False, skip_act=False):
    fpool, bpool, gpool, opool, pspool, dpool = pools

    for half in range(NHALF):
        g = state["g"]
        state["g"] = g + 1
        # SBUF-resident bands for this slice: [p, blk, n, h2], bf16.
        band = bpool.tile([M, NB, NBAND, H2], BF16, tag=f"band{g % 2}")

        for chunk in range(CPH):
            h0 = half * H2 + chunk * NH
            # Contiguous 1.3 MB loads, one per (tensor, channel-half) — small
            # enough that tail DMAs never queue long behind them.
            f1t = fpool.tile([P, 2, NH, W], F32, tag="f1")
            f2t = fpool.tile([P, 2, NH, W], F32, tag="f2")
            for t, src, dst in ((0, f1, f1t), (1, f2, f2t)):
                for cb in range(2):
                    nc.sync.dma_start(
                        out=dst[:, cb].rearrange("p h w -> p (h w)"),
                        in_=src.ap()[cb * P : (cb + 1) * P, h0 : h0 + NH, :],
                    )

            if skip_compute:
                continue
            for hl in range(NH):
                hloc = chunk * NH + hl  # h index within this slice
                ps = pspool.tile([M, NB * NBAND], F32, tag="ps")
                if not skip_mm:
                    for blk in range(NB):
                        w0 = blk * M
                        # rhs covers f2 columns [w0-4, w0+68); edge blocks
                        # clamp to the valid range and leave the out-of-range
                        # band columns for the memsets below.
                        lo = max(0, w0 - D)
                        hi = min(W, w0 + M + D)
                        n0 = lo - (w0 - D)
                        nn = hi - lo
                        for cb in range(2):
                            nc.tensor.matmul(
                                ps[:, blk * NBAND + n0 : blk * NBAND + n0 + nn],
                                f1t[:, cb, hl, w0 : w0 + M],
                                f2t[:, cb, hl, lo:hi],
                                start=(cb == 0),
                                stop=(cb == 1),
                            )
                if not skip_act:
                    # psum (blk, n) -> band[:, blk, n, hloc], scaled to mean.
                    nc.scalar.activation(
                        band[:, :, :, hloc],
                        ps.rearrange("p (b n) -> p b n", b=NB),
                        mybir.ActivationFunctionType.Copy,
                        scale=1.0 / C,
                    )

            # Emit the previous slice's transposes after this slice's last
            # chunk of matmuls: the PE reaches them only once their gather
            # (emitted at the end of the previous slice) has landed.
            if chunk == CPH - 1 and state["pending"] is not None and not skip_gather:
                _transpose_out(nc, pools, out, *state["pending"])
                state["pending"] = None

        if skip_compute:
            continue
        # Out-of-range band columns (w+i-4 outside [0, W)) must be zero.
        nc.vector.memset(band[:, 0, 0:D, :], 0.0)
        nc.vector.memset(band[:, NB - 1, NBAND - D : NBAND, :], 0.0)
        if not skip_gather:
            g9 = _dump_gather(nc, pools, band, g)
            state["pending"] = (g9, half)


def kernel(feat1: np.ndarray, feat2: np.ndarray) -> np.ndarray:
    assert feat1.shape == (B, C, H, W), feat1.shape
    assert feat2.shape == (B, C, H, W), feat2.shape
    if "nc" not in _cache:
        _cache["nc"] = _build()
    nc = _cache["nc"]
    feat1 = np.ascontiguousarray(feat1, dtype=np.float32)
    feat2 = np.ascontiguousarray(feat2, dtype=np.float32)
    in_maps = [{"f1": feat1[b], "f2": feat2[b]} for b in range(B)]
    res = run_bass_kernel_spmd(nc, in_maps, core_ids=list(range(B)))
    return np.stack([res.results[b]["out"] for b in range(B)], axis=0)
